# revision 12
# baseline (speedup 1.0000x reference)
"""Trainium2 Bass kernel: strided 3x3 conv (stride 2, pad 1) + bias
+ hardswish + mish, data-parallel over batch across 8 NeuronCores.

Shapes (hardcoded):
  x (16,64,256,256) f32; weight (128,64,3,3); bias (128,)
  out (16,128,128,128) f32

Design:
- Host pre-pads, de-interleaves and fp16-casts x into [128,257,257]
  per core (2 images x 64ch fused on the leading dim): row 0 = top
  zero pad; per row: [128 even cols | 129 odd cols (leading left-pad
  zero)]. Every conv tap reads a CONTIGUOUS 128-wide slice; each
  supertile x DMA is one ~2.2MB transfer, prefetched one supertile
  ahead on the Sync queue so the PE never starves.
- Conv = 10 fp16 tap-matmuls (fp32 PSUM accumulate) per 512-col PSUM
  slice (9 weight taps + 1 bias tap: (b-0.5)/64 replicated over K=64
  against a ones tile). The two images per core are packed in PE row
  groups (partitions 0-63 / 64-127, tile_position (0,0)/(64,0)) so
  each tap's two matmuls stream concurrently.
- Pointwise tail per chunk (hardswish exact, mish approximated):
    r1b = ACT.Relu(y*(MB/6) + MB/2)            -> MB*hardsigmoid
    hb  = DVE.stt (min(r1b,MB) * y)            -> MB*hardswish(y)
    S   = ACT.Silu(hb*(MK/MB) + MC)
    u1  = DVE.ts  (S*MA + ME)                  (4x mode, fp16)
    out = DVE.tt  (hb + u1)                    (2x mode, fp16)
  where mish(h) ~= MA*silu(MK*h+MC) + MB*h + ME, an LSQ fit over the
  actual h distribution (total fp16-pipeline rel err ~1.0e-3 vs the
  2e-2 gate). Silu+Relu share one ACT table set.
- Output staged fp16 (halves HBM write traffic), upcast on host.
  out_ext is [COUT, PER, HO, WO] so the DMA partition dim is COUT;
  out-DMAs ride the idle GpSimd queue so they never block x loads.
"""
import numpy as np

import concourse.bass as bass
import concourse.mybir as mybir
import concourse.tile as tile
from concourse import bacc
from concourse.bass_utils import run_bass_kernel_spmd

F32 = mybir.dt.float32
F16 = mybir.dt.float16
AFT = mybir.ActivationFunctionType
ALU = mybir.AluOpType

B, CIN, H, W = 16, 64, 256, 256
COUT = 128
HO, WO = 128, 128
NCORE = 8
PER = B // NCORE          # images per core
WP = W + 1                # de-interleaved row width (128 even + 129 odd)
NTAP = 10                 # 9 conv taps + bias tap
NSUP = 8                  # supertiles per core
RS = 16                   # output rows per supertile (2 chunks of 8)
RINS = 2 * RS + 1         # input row slots per supertile (33)

_CACHE: dict = {}

# inner-column offset into the de-interleaved row, per kj
_KJ_OFF = {0: 128, 1: 0, 2: 129}

# mish(h) ~= MA*silu(MK*h+MC) + MB*h + ME (LSQ fit, h = hardswish(y))
MK = 1.55395564
MC = 0.02604102
MA = 0.53451638
MB = 0.17232180
ME = -0.00717160


def _build():
    nc = bacc.Bacc(None, target_bir_lowering=False)
    x_ext = nc.declare_dram_parameter("x", [PER * CIN, H + 1, WP], F16,
                                      isOutput=False)
    wt_ext = nc.declare_dram_parameter("wt", [128, NTAP * COUT], F16,
                                       isOutput=False)
    ones_ext = nc.declare_dram_parameter("ones", [128, 512], F16,
                                         isOutput=False)
    out_ext = nc.declare_dram_parameter("out", [COUT, PER, HO, WO], F16,
                                        isOutput=True)

    with tile.TileContext(nc) as tc:
        with (
            tc.tile_pool(name="const", bufs=1) as cpool,
            tc.tile_pool(name="xin", bufs=3) as xpool,
            tc.tile_pool(name="act", bufs=2) as apool,
            tc.tile_pool(name="psum", bufs=2, space="PSUM") as ppool,
        ):
            wt_sb = cpool.tile([128, NTAP * COUT], F16)
            nc.sync.dma_start(out=wt_sb[:], in_=wt_ext[:])
            ones_sb = cpool.tile([128, 512], F16)
            nc.sync.dma_start(out=ones_sb[:], in_=ones_ext[:])
            hbias_sb = cpool.tile([128, 1], F32)
            nc.vector.memset(hbias_sb[:], 0.5 * MB)
            mc_sb = cpool.tile([128, 1], F32)
            nc.vector.memset(mc_sb[:], MC)

            # HAM warmup: ~6us of dummy matmuls so the PE clock is at
            # 2.4GHz for the real work; overlaps the first x DMA.
            warm = ppool.tile([128, 2048], F32, tag="pt", name="warm")
            for m in range(28):
                p0 = 64 * (m % 2)
                nc.tensor.matmul(
                    warm[:, (m % 4) * 512 : (m % 4) * 512 + 512],
                    wt_sb[p0 : p0 + 64, 9 * COUT : 10 * COUT],
                    ones_sb[p0 : p0 + 64, :],
                    start=True, stop=True, tile_position=(p0, 0),
                )
            # consume the scratch so nothing is left write-only
            wsink = cpool.tile([128, 8], F32)
            nc.scalar.activation(wsink[:], warm[:, 0:8], AFT.Identity)

            N1 = 8 * WO            # 1024: one image-chunk (8 out rows)

            def load(st):
                xt = xpool.tile([128, RINS * WP], F16, name="xt")
                xt3 = xt[:].rearrange("p (r c) -> p r c", c=WP)
                nc.sync.dma_start(
                    out=xt3[:, :, :],
                    in_=x_ext[:, 2 * RS * st : 2 * RS * st + RINS, :],
                )
                return xt3

            def stage2(hb, c):
                # mish fit from hb = MB*hardswish; out rows 8c..8c+8
                ss = apool.tile([128, 2048], F16, name="ss")
                nc.scalar.activation(ss[:], hb[:], AFT.Silu,
                                     scale=MK / MB, bias=mc_sb[:, 0:1])
                u1 = apool.tile([128, 2048], F16, name="u1")
                nc.vector.tensor_scalar(u1[:], ss[:], MA, ME,
                                        ALU.mult, ALU.add)
                te = apool.tile([128, 2048], F16, name="te")
                nc.vector.tensor_tensor(te[:], hb[:], u1[:], ALU.add)
                rg0 = 8 * c
                nc.gpsimd.dma_start(
                    out=out_ext[:, :, rg0 : rg0 + 8, :],
                    in_=te[:].rearrange("p (i r w) -> p i r w",
                                        i=PER, w=WO),
                )

            xts = {0: load(0), 1: load(1)}
            pend = None
            for c in range(2 * NSUP):
                st, cl = divmod(c, 2)
                if cl == 0 and st + 2 < NSUP:
                    xts[st + 2] = load(st + 2)
                xt3 = xts[st]
                pts = ppool.tile([128, 2048], F32, tag="pt", name="pt")
                for g in range(2):
                    gg = 2 * cl + g
                    for t in [9] + list(range(9)):
                        for i in range(PER):
                            p0 = 64 * i
                            if t == 9:  # bias tap
                                rhs = ones_sb[p0 : p0 + 64, :]
                            else:
                                ki, kj = divmod(t, 3)
                                s = 8 * gg + ki
                                off = _KJ_OFF[kj]
                                rhs = xt3[p0 : p0 + 64, s : s + 7 : 2,
                                          off : off + WO]
                            lhsT = wt_sb[p0 : p0 + 64,
                                         t * COUT : (t + 1) * COUT]
                            nc.tensor.matmul(
                                pts[:, i * N1 + g * 512
                                    : i * N1 + g * 512 + 512],
                                lhsT, rhs,
                                start=(t == 9), stop=(t == 8),
                                tile_position=(p0, 0),
                            )
                # stage-2 of the previous chunk runs on ACT/DVE while this
                # chunk's matmuls stream (sw pipelining keeps the in-order
                # ACT queue from serializing relu(c) behind silu(c)).
                if pend is not None:
                    stage2(*pend)
                # ---- stage-1: exact hardswish (scaled by MB) ----
                r1 = apool.tile([128, 2048], F32, name="r1")
                nc.scalar.activation(r1[:], pts[:], AFT.Relu,
                                     scale=MB / 6.0,
                                     bias=hbias_sb[:, 0:1])
                hb = apool.tile([128, 2048], F16, name="hb")
                nc.vector.scalar_tensor_tensor(hb[:], r1[:], MB,
                                               pts[:], ALU.min, ALU.mult)
                pend = (hb, c)
            stage2(*pend)
    nc.compile()
    return nc


def _get_nc():
    if "nc" not in _CACHE:
        _CACHE["nc"] = _build()
    return _CACHE["nc"]


def _prep(x, weight, bias):
    x = np.asarray(x, dtype=np.float32)
    w = np.asarray(weight, dtype=np.float32)
    b = np.asarray(bias, dtype=np.float32)

    # de-interleave + pad + fp16: row 0 = top pad; cols [0:128]=even orig
    # cols, [128]=left pad, [129:257]=odd orig cols 1,3,...,255
    x_de = np.zeros((B, CIN, H + 1, WP), dtype=np.float16)
    x_de[:, :, 1:, 0:128] = x[:, :, :, 0::2]
    x_de[:, :, 1:, 129:257] = x[:, :, :, 1::2]
    x_de = x_de.reshape(NCORE, PER * CIN, H + 1, WP)

    # wt: [cin, tap*COUT]; tap 9 = (bias-0.5)/64 replicated over cin;
    # duplicated across both partition halves
    wt = np.empty((CIN, NTAP * COUT), dtype=np.float16)
    wt[:, : 9 * COUT] = w.transpose(1, 2, 3, 0).reshape(CIN, 9 * COUT)
    wt[:, 9 * COUT :] = ((b.astype(np.float64) - 0.5) / 64.0)[None, :]
    wt2 = np.ascontiguousarray(np.concatenate([wt, wt], axis=0))

    ones = np.ones((128, 512), dtype=np.float16)
    in_maps = [
        {"x": x_de[i], "wt": wt2, "ones": ones}
        for i in range(NCORE)
    ]
    return in_maps


def _run(in_maps, **kw):
    nc = _get_nc()
    return run_bass_kernel_spmd(nc, in_maps, list(range(NCORE)), **kw)


def kernel(x, weight, bias):
    res = _run(_prep(x, weight, bias))
    # out is [COUT, PER, HO, WO] fp16 per core -> [PER, COUT, HO, WO] f32
    outs = [res.results[i]["out"].transpose(1, 0, 2, 3) for i in range(NCORE)]
    return np.concatenate(outs, axis=0).astype(np.float32)


# revision 15
# speedup vs baseline: 1.0107x; 1.0107x over previous
"""Trainium2 Bass kernel: strided 3x3 conv (stride 2, pad 1) + bias
+ hardswish + mish, data-parallel over batch across 8 NeuronCores.

Shapes (hardcoded):
  x (16,64,256,256) f32; weight (128,64,3,3); bias (128,)
  out (16,128,128,128) f32

Design:
- Host pre-pads, de-interleaves and fp16-casts x into [128,257,257]
  per core (2 images x 64ch fused on the leading dim): row 0 = top
  zero pad; per row: [128 even cols | 129 odd cols (leading left-pad
  zero)]. Every conv tap reads a CONTIGUOUS 128-wide slice; each
  supertile x DMA is one ~2.2MB transfer, prefetched one supertile
  ahead on the Sync queue so the PE never starves.
- Conv = 10 fp16 tap-matmuls (fp32 PSUM accumulate) per 512-col PSUM
  slice (9 weight taps + 1 bias tap: (b-0.5)/64 replicated over K=64
  against a ones tile). The two images per core are packed in PE row
  groups (partitions 0-63 / 64-127, tile_position (0,0)/(64,0)) so
  each tap's two matmuls stream concurrently.
- Pointwise tail per chunk (hardswish exact, mish approximated):
    r1b = ACT.Relu(y*(MB/6) + MB/2)            -> MB*hardsigmoid
    hb  = DVE.stt (min(r1b,MB) * y)            -> MB*hardswish(y)
    S   = ACT.Silu(hb*(MK/MB) + MC)
    u1  = DVE.ts  (S*MA + ME)                  (4x mode, fp16)
    out = DVE.tt  (hb + u1)                    (2x mode, fp16)
  where mish(h) ~= MA*silu(MK*h+MC) + MB*h + ME, an LSQ fit over the
  actual h distribution (total fp16-pipeline rel err ~1.0e-3 vs the
  2e-2 gate). Silu+Relu share one ACT table set.
- Output staged fp16 (halves HBM write traffic), upcast on host.
  out_ext is [COUT, PER, HO, WO] so the DMA partition dim is COUT;
  out-DMAs ride the idle GpSimd queue so they never block x loads.
"""
import numpy as np

import concourse.bass as bass
import concourse.mybir as mybir
import concourse.tile as tile
from concourse import bacc
from concourse.bass_utils import run_bass_kernel_spmd

F32 = mybir.dt.float32
F16 = mybir.dt.float16
AFT = mybir.ActivationFunctionType
ALU = mybir.AluOpType

B, CIN, H, W = 16, 64, 256, 256
COUT = 128
HO, WO = 128, 128
NCORE = 8
PER = B // NCORE          # images per core
WP = W + 1                # de-interleaved row width (128 even + 129 odd)
NTAP = 10                 # 9 conv taps + bias tap
NSUP = 8                  # supertiles per core
RS = 16                   # output rows per supertile (2 chunks of 8)
RINS = 2 * RS + 1         # input row slots per supertile (33)

_CACHE: dict = {}

# inner-column offset into the de-interleaved row, per kj
_KJ_OFF = {0: 128, 1: 0, 2: 129}

# mish(h) ~= MA*silu(MK*h+MC) + MB*h + ME (LSQ fit, h = hardswish(y))
MK = 1.55395564
MC = 0.02604102
MA = 0.53451638
MB = 0.17232180
ME = -0.00717160


def _build():
    nc = bacc.Bacc(None, target_bir_lowering=False)
    x_ext = nc.declare_dram_parameter("x", [PER * CIN, H + 1, WP], F16,
                                      isOutput=False)
    wt_ext = nc.declare_dram_parameter("wt", [128, NTAP * COUT], F16,
                                       isOutput=False)
    ones_ext = nc.declare_dram_parameter("ones", [128, 512], F16,
                                         isOutput=False)
    out_ext = nc.declare_dram_parameter("out", [COUT, PER, HO, WO], F16,
                                        isOutput=True)

    with tile.TileContext(nc) as tc:
        with (
            tc.tile_pool(name="const", bufs=1) as cpool,
            tc.tile_pool(name="xin", bufs=3) as xpool,
            tc.tile_pool(name="act", bufs=2) as apool,
            tc.tile_pool(name="hbp", bufs=3) as hpool,
            tc.tile_pool(name="psum", bufs=2, space="PSUM") as ppool,
        ):
            wt_sb = cpool.tile([128, NTAP * COUT], F16)
            nc.sync.dma_start(out=wt_sb[:], in_=wt_ext[:])
            ones_sb = cpool.tile([128, 512], F16)
            nc.sync.dma_start(out=ones_sb[:], in_=ones_ext[:])
            hbias_sb = cpool.tile([128, 1], F32)
            nc.vector.memset(hbias_sb[:], 0.5 * MB)
            mc_sb = cpool.tile([128, 1], F32)
            nc.vector.memset(mc_sb[:], MC)

            # HAM warmup: ~6us of dummy matmuls so the PE clock is at
            # 2.4GHz for the real work; overlaps the first x DMA.
            warm = ppool.tile([128, 2048], F32, tag="pt", name="warm")
            for m in range(28):
                p0 = 64 * (m % 2)
                nc.tensor.matmul(
                    warm[:, (m % 4) * 512 : (m % 4) * 512 + 512],
                    wt_sb[p0 : p0 + 64, 9 * COUT : 10 * COUT],
                    ones_sb[p0 : p0 + 64, :],
                    start=True, stop=True, tile_position=(p0, 0),
                )
            # consume the scratch so nothing is left write-only
            wsink = cpool.tile([128, 8], F32)
            nc.scalar.activation(wsink[:], warm[:, 0:8], AFT.Identity)

            N1 = 8 * WO            # 1024: one image-chunk (8 out rows)

            def load(st):
                xt = xpool.tile([128, RINS * WP], F16, name="xt")
                xt3 = xt[:].rearrange("p (r c) -> p r c", c=WP)
                nc.sync.dma_start(
                    out=xt3[:, :, :],
                    in_=x_ext[:, 2 * RS * st : 2 * RS * st + RINS, :],
                )
                return xt3

            def stage2(hb, c):
                # mish fit from hb = MB*hardswish; out rows 8c..8c+8
                ss = apool.tile([128, 2048], F16, name="ss")
                nc.scalar.activation(ss[:], hb[:], AFT.Silu,
                                     scale=MK / MB, bias=mc_sb[:, 0:1])
                u1 = apool.tile([128, 2048], F16, name="u1")
                nc.vector.tensor_scalar(u1[:], ss[:], MA, ME,
                                        ALU.mult, ALU.add)
                te = apool.tile([128, 2048], F16, name="te")
                nc.vector.tensor_tensor(te[:], hb[:], u1[:], ALU.add)
                rg0 = 8 * c
                nc.gpsimd.dma_start(
                    out=out_ext[:, :, rg0 : rg0 + 8, :],
                    in_=te[:].rearrange("p (i r w) -> p i r w",
                                        i=PER, w=WO),
                )

            xts = {0: load(0), 1: load(1)}
            pend = []
            for c in range(2 * NSUP):
                st, cl = divmod(c, 2)
                if cl == 0 and st + 2 < NSUP:
                    xts[st + 2] = load(st + 2)
                xt3 = xts[st]
                pts = ppool.tile([128, 2048], F32, tag="pt", name="pt")
                for g in range(2):
                    gg = 2 * cl + g
                    for t in [9] + list(range(9)):
                        for i in range(PER):
                            p0 = 64 * i
                            if t == 9:  # bias tap
                                rhs = ones_sb[p0 : p0 + 64, :]
                            else:
                                ki, kj = divmod(t, 3)
                                s = 8 * gg + ki
                                off = _KJ_OFF[kj]
                                rhs = xt3[p0 : p0 + 64, s : s + 7 : 2,
                                          off : off + WO]
                            lhsT = wt_sb[p0 : p0 + 64,
                                         t * COUT : (t + 1) * COUT]
                            nc.tensor.matmul(
                                pts[:, i * N1 + g * 512
                                    : i * N1 + g * 512 + 512],
                                lhsT, rhs,
                                start=(t == 9), stop=(t == 8),
                                tile_position=(p0, 0),
                            )
                # stage-2 runs TWO chunks behind: the in-order ACT queue
                # becomes [..., silu(c-2), relu(c), silu(c-1), ...] so
                # every ACT op's input is ready when the engine reaches it
                # (silu(c) directly after relu(c) would serialize the
                # relu->hb->silu cross-engine chain at 6.2us/chunk).
                if len(pend) == 2:
                    stage2(*pend.pop(0))
                # ---- stage-1: exact hardswish (scaled by MB) ----
                r1 = apool.tile([128, 2048], F32, name="r1")
                nc.scalar.activation(r1[:], pts[:], AFT.Relu,
                                     scale=MB / 6.0,
                                     bias=hbias_sb[:, 0:1])
                hb = hpool.tile([128, 2048], F16, name="hb")
                nc.vector.scalar_tensor_tensor(hb[:], r1[:], MB,
                                               pts[:], ALU.min, ALU.mult)
                pend.append((hb, c))
            for p in pend:
                stage2(*p)
    nc.compile()
    return nc


def _get_nc():
    if "nc" not in _CACHE:
        _CACHE["nc"] = _build()
    return _CACHE["nc"]


def _prep(x, weight, bias):
    x = np.asarray(x, dtype=np.float32)
    w = np.asarray(weight, dtype=np.float32)
    b = np.asarray(bias, dtype=np.float32)

    # de-interleave + pad + fp16: row 0 = top pad; cols [0:128]=even orig
    # cols, [128]=left pad, [129:257]=odd orig cols 1,3,...,255
    x_de = np.zeros((B, CIN, H + 1, WP), dtype=np.float16)
    x_de[:, :, 1:, 0:128] = x[:, :, :, 0::2]
    x_de[:, :, 1:, 129:257] = x[:, :, :, 1::2]
    x_de = x_de.reshape(NCORE, PER * CIN, H + 1, WP)

    # wt: [cin, tap*COUT]; tap 9 = (bias-0.5)/64 replicated over cin;
    # duplicated across both partition halves
    wt = np.empty((CIN, NTAP * COUT), dtype=np.float16)
    wt[:, : 9 * COUT] = w.transpose(1, 2, 3, 0).reshape(CIN, 9 * COUT)
    wt[:, 9 * COUT :] = ((b.astype(np.float64) - 0.5) / 64.0)[None, :]
    wt2 = np.ascontiguousarray(np.concatenate([wt, wt], axis=0))

    ones = np.ones((128, 512), dtype=np.float16)
    in_maps = [
        {"x": x_de[i], "wt": wt2, "ones": ones}
        for i in range(NCORE)
    ]
    return in_maps


def _run(in_maps, **kw):
    nc = _get_nc()
    return run_bass_kernel_spmd(nc, in_maps, list(range(NCORE)), **kw)


def kernel(x, weight, bias):
    res = _run(_prep(x, weight, bias))
    # out is [COUT, PER, HO, WO] fp16 per core -> [PER, COUT, HO, WO] f32
    outs = [res.results[i]["out"].transpose(1, 0, 2, 3) for i in range(NCORE)]
    return np.concatenate(outs, axis=0).astype(np.float32)
